# revision 11
# baseline (speedup 1.0000x reference)
"""CP-decomposed conv (pointwise -> depthwise-h -> depthwise-w -> pointwise)
as a Bass/Tile kernel on 8 TRN2 NeuronCores.

Strategy:
  - Data-parallel over batch: 32 images -> 4 per core, no collectives.
  - fp16 wire format: x and out cross HBM as fp16; fp32 accumulation in PSUM.
  - Stage A+B: pointwise C->R with the depthwise h-conv folded in:
      y2[r,i,w] = sum_{h,c} (factor3[c,r]*factor1[h,r]) * x[c,i+h,w]
    6 accumulating fp16 matmuls per 1-bank PSUM tile (3 h-shifts x 2
    C-chunks); psA bufs=4 gives the PE three tiles of slack over stage C.
  - Stage C: depthwise w-conv normalized by factor2[0,:] (folded into the
    stage-D weights) so tap0 is a plain ACT copy; taps 1-2 are DVE STTs
    (the ISA allows at most one PSUM input per elementwise op):
      y3' = pa[0] + (f2[1]/f2[0])*pa[1] + (f2[2]/f2[0])*pa[2]
  - Stage D: projection R->F with weights factor0[f,r]*factor2[0,r]; psD
    spans 2 banks so each PSUM->SBUF copy moves 1024 elements. Copies
    split ~30/70 between DVE and ACT to balance the engines.
  - DMA: weights preshuffled on host into a contiguous per-partition
    layout (128 descriptors instead of 1280 tiny ones). Whole half-image
    input loads (2.4 MB, SWDGE on GpSimd); the very first load is split
    into row chunks so the PE starts after ~1 chunk. One output DMA per
    (half-image, f-chunk) (1.13 MB, HWDGE on SP).
  - Stage-D chunk units of half h-1 are interleaved two-per-tile between
    the stage-A tiles of half h, so the PE queue always holds
    dependency-free work and the copies spread across the whole half.
"""

import sys
import numpy as np

for _p in ("/opt/trn_rl_repo",):
    if _p not in sys.path:
        sys.path.insert(0, _p)

B, C, H, W = 32, 256, 96, 96
F, FH, FW, R = 512, 3, 3, 128
OH, OW = H - FH + 1, W - FW + 1  # 94, 94
NCORES = 8
BLOC = B // NCORES  # 4 images per core

SH = 47            # output rows per half-image
ROW_TILES = [(r0, min(5, SH - r0)) for r0 in range(0, SH, 5)]
COL_CHUNKS = [(0, 1024), (1024, 1024), (2048, 1024), (3072, 1024), (4096, 322)]
# row chunks for the first input load (output-row tiles r0..r0+nr need x
# rows r0..r0+nr+2); non-overlapping x-row ranges covering the full image
FIRST_XCHUNKS = [(0, 7), (7, 10), (17, 10), (27, 10), (37, 12), (49, 47)]

_NC_CACHE = {}


def _build_nc():
    import concourse.bacc as bacc
    import concourse.mybir as mybir
    import concourse.tile as tile

    f32 = mybir.dt.float32
    f16 = mybir.dt.float16
    mult = mybir.AluOpType.mult
    add = mybir.AluOpType.add

    nc = bacc.Bacc("TRN2", target_bir_lowering=False, debug=True)

    xd = nc.dram_tensor("x", [BLOC, C, H, W], f16, kind="ExternalInput")
    # wab: [p, t, c] per-partition-major packed weights (contiguous DMA):
    #   t in 0..5  -> stage-A tiles [c', r] = factor3[ch*128+c',r]*factor1[h,r]
    #   t in 6..9  -> stage-D tiles [r, f'] = factor0[fc*128+f',r]*factor2[0,r]
    wabd = nc.dram_tensor("wab", [128, 10, 128], f16, kind="ExternalInput")
    # wc: [r, j] = factor2[j+1, r] / factor2[0, r]
    wcd = nc.dram_tensor("wc", [R, 2], f32, kind="ExternalInput")
    od = nc.dram_tensor("out", [BLOC, F, OH, OW], f16, kind="ExternalOutput")

    with tile.TileContext(nc) as tc:
        with (
            tc.tile_pool(name="wpool", bufs=1) as wpool,
            tc.tile_pool(name="xs", bufs=3) as xs_pool,
            tc.tile_pool(name="y3", bufs=3) as y3_pool,
            tc.tile_pool(name="osb", bufs=2) as osb_pool,
            tc.tile_pool(name="psA", bufs=4, space="PSUM") as psA,
            tc.tile_pool(name="psD", bufs=2, space="PSUM") as psD,
        ):
            wab_sb = wpool.tile([128, 10, 128], f16)
            nc.sync.dma_start(wab_sb[:], wabd[:])
            wc_sb = wpool.tile([128, 2], f32)
            nc.sync.dma_start(wc_sb[:], wcd[:])
            wb_off = FH * 2  # wab_sb[:, wb_off+fc, :] for stage D

            copy_i = 0  # stage-D PSUM->SBUF copy split: ~30% DVE, 70% ACT

            def psum_copy(dst, src, drain=False):
                nonlocal copy_i
                if drain:
                    use_dve = copy_i % 2 == 0  # both engines idle: alternate
                else:
                    use_dve = copy_i % 10 in (0, 4, 7)
                if use_dve:
                    nc.vector.tensor_copy(dst, src)
                else:
                    nc.scalar.copy(dst, src)
                copy_i += 1

            def emit_a_tile(xs_t, y3_t, half, r0, nr):
                """Stage A+B matmuls and 3-op stage C for one 5-row tile."""
                pa = psA.tile([128, 512], f32)
                k = 0
                for h in range(FH):
                    for ch in range(2):
                        x0 = (half * SH + r0 + h) * W
                        nc.tensor.matmul(
                            pa[:, 0 : nr * W],
                            wab_sb[:, h * 2 + ch, :],
                            xs_t[:, ch, x0 : x0 + nr * W],
                            start=(k == 0),
                            stop=(k == 5),
                        )
                        k += 1
                # w-conv: at most one PSUM input per elementwise op -> 3 ops.
                # tap0 is a plain ACT copy (factor2[0,:] is folded into the
                # stage-D weights); taps 1-2 are DVE STTs accumulating on it.
                s3 = pa[:, 0 : nr * W].rearrange("p (r c) -> p r c", c=W)
                dst = y3_t[:, r0 * OW : (r0 + nr) * OW]
                nc.scalar.copy(dst, s3[:, :, 0:OW])
                nc.vector.scalar_tensor_tensor(
                    dst, s3[:, :, 1 : 1 + OW], wc_sb[:, 0:1], dst,
                    op0=mult, op1=add,
                )
                nc.vector.scalar_tensor_tensor(
                    dst, s3[:, :, 2 : 2 + OW], wc_sb[:, 2 - 1 : 2], dst,
                    op0=mult, op1=add,
                )

            def d_unit_gen(b, half, y3_t, drain=False):
                """Yield stage-D emitters: 20 (fc, col-chunk) units. Each
                fc's output DMA goes out right after its last chunk; in
                drain mode it is split in two so it starts earlier."""
                i0 = half * SH
                ot = osb_pool.tile([128, 4, SH * OW], f16)
                for fc in range(4):
                    od_flat = od[b, fc * 128 : (fc + 1) * 128,
                                 i0 : i0 + SH, :].rearrange("p r c -> p (r c)")
                    for ci, (c0, cw) in enumerate(COL_CHUNKS):
                        def unit(fc=fc, ci=ci, c0=c0, cw=cw, od_flat=od_flat):
                            pd = psD.tile([128, 2, 512], f32)
                            for j in range((cw + 511) // 512):
                                w_ = min(512, cw - j * 512)
                                nc.tensor.matmul(
                                    pd[:, j, 0:w_],
                                    wab_sb[:, wb_off + fc, :],
                                    y3_t[:, c0 + j * 512 : c0 + j * 512 + w_],
                                    start=True,
                                    stop=True,
                                )
                            src = pd[:, :, :] if cw > 512 else pd[:, 0, 0:cw]
                            psum_copy(ot[:, fc, c0 : c0 + cw], src, drain)
                            if drain and ci == 1:
                                nc.sync.dma_start(
                                    od_flat[:, 0:2048], ot[:, fc, 0:2048]
                                )
                            if ci == len(COL_CHUNKS) - 1:
                                if drain:
                                    nc.sync.dma_start(
                                        od_flat[:, 2048 : SH * OW],
                                        ot[:, fc, 2048 : SH * OW],
                                    )
                                else:
                                    nc.sync.dma_start(
                                        od_flat[:], ot[:, fc, :]
                                    )
                        yield unit

            def load_xs(b, chunks=None):
                xs_t = xs_pool.tile([128, 2, H * W], f16)
                if chunks is None:
                    chunks = [(0, H)]
                for x0, xn in chunks:
                    nc.gpsimd.dma_start(
                        xs_t[:, :, x0 * W : (x0 + xn) * W],
                        xd[b, :, x0 : x0 + xn, :]
                        .rearrange("(t p) r c -> p t (r c)", p=128),
                    )
                return xs_t

            # software pipeline: stage-D chunk units of half h-1 are
            # interleaved two-per-tile between the stage-A tiles of half h,
            # so the PE queue always has dependency-free work and the
            # PSUM->SBUF copies are spread across the whole half.
            dgen = iter(())  # stage-D units of the previous half
            for b in range(BLOC):
                xs_t = load_xs(b, FIRST_XCHUNKS if b == 0 else None)
                for half in range(2):
                    y3_t = y3_pool.tile([128, SH * OW], f16)
                    for r0, nr in ROW_TILES:
                        emit_a_tile(xs_t, y3_t, half, r0, nr)
                        for _ in range(2):
                            u = next(dgen, None)
                            if u is not None:
                                u()
                    for u in dgen:  # drain any leftovers
                        u()
                    dgen = d_unit_gen(
                        b, half, y3_t,
                        drain=(b == BLOC - 1 and half == 1),
                    )
            for u in dgen:  # stage D of the final half
                u()

    nc.compile()
    return nc


def _get_nc():
    if "nc" not in _NC_CACHE:
        _NC_CACHE["nc"] = _build_nc()
    return _NC_CACHE["nc"]


def _prep_weights(factor0, factor1, factor2, factor3):
    s0 = factor2[0]  # [R] normalization tap (folded into stage-D weights)
    # stage-A tiles: [c', t=h*2+ch, r]
    wa = (factor3[None, :, :] * factor1[:, None, :]).reshape(FH, 2, 128, R)
    wa = wa.transpose(2, 0, 1, 3).reshape(128, 6, R)  # [c', (h,ch), r]
    # stage-D tiles: [r, t=fc, f'] = factor0[fc*128+f', r] * s0[r]
    wb = (factor0 * s0[None, :]).reshape(4, 128, R)
    wb = wb.transpose(2, 0, 1)  # [r, fc, f']
    wab = np.concatenate([wa, wb], axis=1).astype(np.float16)
    wab = np.ascontiguousarray(wab)  # [128, 10, 128]
    # wc[r, j] = factor2[j+1, r] / factor2[0, r]
    wc = np.ascontiguousarray(
        (factor2[1:] / s0[None, :]).T, dtype=np.float32
    )
    return wab, wc


def _prep_x(x):
    return np.ascontiguousarray(x).astype(np.float16)


def kernel(x, factor0, factor1, factor2, factor3):
    from concourse import bass_utils

    x = np.asarray(x, dtype=np.float32)
    factor0 = np.asarray(factor0, dtype=np.float32)
    factor1 = np.asarray(factor1, dtype=np.float32)
    factor2 = np.asarray(factor2, dtype=np.float32)
    factor3 = np.asarray(factor3, dtype=np.float32)

    wab, wc = _prep_weights(factor0, factor1, factor2, factor3)
    x16 = _prep_x(x)

    nc = _get_nc()
    in_maps = [
        {"x": x16[c * BLOC : (c + 1) * BLOC], "wab": wab, "wc": wc}
        for c in range(NCORES)
    ]
    res = bass_utils.run_bass_kernel_spmd(nc, in_maps, list(range(NCORES)))
    out = np.concatenate(
        [res.results[c]["out"] for c in range(NCORES)], axis=0
    )
    return out.astype(np.float32)
